# revision 14
# baseline (speedup 1.0000x reference)
"""Trainium2 Bass kernel for nn_BarrierPolicy: per-sample MLP + closed-form QP.

Data-parallel over 8 NeuronCores: each core processes 32768 samples.

Per-core pipeline (feature-major mainline, batch-major epilogue):
  - x is loaded batch-major ("p-major" sample mapping: sample = p*256 + j),
    PE-transposed in [128,32] groups, bounced through DRAM to coalesce into
    feature-major [8, 4096] tiles (512-sample matmul blocks).
  - L1/L2/L3 matmuls stream float32r (1 cycle/row) with weights stationary.
  - ReLU+bias fused into PSUM->SBUF moves on ACT/DVE.
  - Heads (w31/w32) computed with activations-stationary tiny matmuls so the
    outputs land batch-major [128 samples, cols] directly in PSUM.
  - QP closed form evaluated batch-major on DVE, output written densely.
"""

import os
import sys

import numpy as np

for _p in ("/opt/trn_rl_repo", os.path.expanduser("~/.axon_site/_ro/trn_rl_repo")):
    if os.path.isdir(_p) and _p not in sys.path:
        sys.path.append(_p)

import concourse.bacc as bacc
import concourse.mybir as mybir
import concourse.tile as tile
from concourse.bass_utils import run_bass_kernel_spmd

dt = mybir.dt
AF = mybir.ActivationFunctionType
ALU = mybir.AluOpType

N_CORES = 8
B_FULL, F, H1, H2, C = 262144, 8, 128, 128, 2
BS = B_FULL // N_CORES  # 32768 samples per core
P = 128                 # SBUF partitions
J = BS // P             # 256 samples per partition ("j" index)
NCHUNK = 8              # chunks per core
JC = J // NCHUNK        # 32 j's per chunk (4096 samples)
NBLK = 8                # 512-sample matmul blocks per chunk
BLK = 512
OBS_X, OBS_Y, RADIUS = 40.0, 15.0, 6.0
EPS = 1e-12

# const block column layout (one [128, CW] f32 tensor)
C_LT2 = 0        # w21.T           [128, 128]
C_LT3 = 128      # w22.T           [128, 128]
C_ID = 256       # identity        [128, 128]
C_LT1 = 384      # (w1*std).T      [8, 128] (rows 0:8)
C_W31 = 512      # w31.T           [128, 2]
C_W32 = 514      # w32.T           [128, 2]
C_B1 = 516       # b1 + w1@mean    [128, 1]
C_B21 = 517      # b21             [128, 1]
C_B22 = 518      # b22             [128, 1]
C_EB31 = 519     # b31 pair-tiled  [128, 64]
C_EM01 = 583     # mean01-obs pair [128, 64]
C_ES01 = 647     # std01 pair      [128, 64]
C_EB32 = 711     # b32[0]          [128, 32]
CW = 743


def build_program(repeat=1):
    nc = bacc.Bacc("TRN2", target_bir_lowering=False, debug=False,
                   num_devices=N_CORES)
    f32, f32r = dt.float32, dt.float32r

    x_d = nc.dram_tensor("x", [BS, F], f32r, kind="ExternalInput").ap()
    cst_d = nc.dram_tensor("consts", [P, CW], f32r, kind="ExternalInput").ap()
    out_d = nc.dram_tensor("out", [BS, C], f32, kind="ExternalOutput").ap()
    # DRAM bounce buffers for the fm-coalescing shuffle (one per chunk)
    bnc_d = nc.dram_tensor("bounce", [NCHUNK, 64, BLK], f32r).ap()

    x_r = x_d.rearrange("(p j) f -> p j f", j=J)      # [128, 256, 8]
    out_r = out_d.rearrange("(p j) c -> p j c", j=J)  # [128, 256, 2]

    with tile.TileContext(nc) as tc:
        with (
            tc.tile_pool(name="cstp", bufs=1) as cstp,
            tc.tile_pool(name="xinp", bufs=2) as xinp,
            tc.tile_pool(name="s64p", bufs=2) as s64p,
            tc.tile_pool(name="xtp", bufs=2) as xtp,
            tc.tile_pool(name="h1p", bufs=2) as h1sp,
            tc.tile_pool(name="x2p", bufs=2) as x2p,
            tc.tile_pool(name="epp", bufs=2) as epp,
            tc.tile_pool(name="tpps", bufs=2, space="PSUM") as tpps,
            tc.tile_pool(name="mm1ps", bufs=1, space="PSUM") as mm1ps,
            tc.tile_pool(name="mm2ps", bufs=2, space="PSUM") as mm2ps,
            tc.tile_pool(name="mm3ps", bufs=2, space="PSUM") as mm3ps,
            tc.tile_pool(name="psps", bufs=1, space="PSUM") as psps,
        ):
            cst = cstp.tile([P, CW], f32r)
            nc.sync.dma_start(cst[:], cst_d[:])

            lt1 = cst[0:F, C_LT1:C_LT1 + H1]
            lt2 = cst[:, C_LT2:C_LT2 + H2]
            lt3 = cst[:, C_LT3:C_LT3 + H2]
            ident = cst[:, C_ID:C_ID + 128]
            w31 = cst[:, C_W31:C_W31 + 2]
            w32 = cst[:, C_W32:C_W32 + 2]
            b1 = cst[:, C_B1:C_B1 + 1].bitcast(f32)
            b21 = cst[:, C_B21:C_B21 + 1].bitcast(f32)
            b22 = cst[:, C_B22:C_B22 + 1].bitcast(f32)
            eb31 = cst[:, C_EB31:C_EB31 + 2 * JC].bitcast(f32).rearrange("p (j c) -> p j c", c=2)
            em01 = cst[:, C_EM01:C_EM01 + 2 * JC].bitcast(f32).rearrange("p (j c) -> p j c", c=2)
            es01 = cst[:, C_ES01:C_ES01 + 2 * JC].bitcast(f32).rearrange("p (j c) -> p j c", c=2)
            eb32 = cst[:, C_EB32:C_EB32 + JC].bitcast(f32)

            for _rep in range(repeat):
                for q in range(NCHUNK):
                    # ---- load x chunk, batch-major, p-major sample map ----
                    xc = xinp.tile([P, JC * F], f32r, tag="xc")
                    nc.sync.dma_start(xc[:], x_r[:, q * JC:(q + 1) * JC, :])

                    # ---- PE transpose in [128,32] groups (2 half-chunks) ----
                    # Half h: 4 transposes -> psum [32, 512] (rows jj*8+f,
                    # cols cb*128+p), copy to SBUF, scatter to DRAM bounce
                    # laid out as bounce[f, (h jj cb p)] in 2KB runs.
                    # fm block b = h*4+jj holds samples j_local = 16h+4cb+jj.
                    for h in range(2):
                        tp = tpps.tile([32, BLK], f32r, tag="tp")
                        for cb in range(4):
                            t = 4 * h + cb
                            nc.tensor.transpose(
                                tp[:, 128 * cb:128 * cb + 128],
                                xc[:, 32 * t:32 * t + 32],
                                ident,
                            )
                        s32 = s64p.tile([32, BLK], f32r, tag="s32")
                        nc.vector.tensor_copy(s32[:], tp[:])
                        nc.sync.dma_start(bnc_d[q][32 * h:32 * h + 32, :], s32[:])
                    xt = xtp.tile([F, NBLK * BLK], f32r, tag="xt")  # [8, 4096]
                    rsrc = bnc_d[q].rearrange("(b f) cbp -> f b cbp", f=F)
                    rdst = xt[:].rearrange("f (b cbp) -> f b cbp", b=NBLK)
                    nc.sync.dma_start(rdst, rsrc)

                    ps = psps.tile([P, 4 * JC], f32, tag="ps")  # [128, 128]
                    for b in range(NBLK):
                        h, jj = b // 4, b % 4
                        rhs = xt[:, 2048 * h + BLK * jj:
                                 2048 * h + BLK * (jj + 1)]
                        h1pre = mm1ps.tile([P, BLK], f32, tag="h1pre")
                        nc.tensor.matmul(h1pre[:], lt1, rhs, start=True, stop=True)
                        h1s = h1sp.tile([P, BLK], f32r, tag="h1s")
                        nc.scalar.activation(h1s[:], h1pre[:], AF.Relu, bias=b1)
                        h1r = h1s[:]

                        z21 = mm2ps.tile([P, BLK], f32, tag="z21")
                        z22 = mm3ps.tile([P, BLK], f32, tag="z22")
                        nc.tensor.matmul(z21[:], lt2, h1r, start=True, stop=True)
                        nc.tensor.matmul(z22[:], lt3, h1r, start=True, stop=True)
                        x21 = x2p.tile([P, BLK], f32r, tag="x21")
                        x22 = x2p.tile([P, BLK], f32r, tag="x22")
                        if b % 2 == 0:
                            nc.vector.tensor_scalar(
                                x21[:], z21[:], b21, 0.0, ALU.add, ALU.max)
                            nc.scalar.activation(
                                x22[:], z22[:], AF.Relu, bias=b22)
                        else:
                            nc.scalar.activation(
                                x21[:], z21[:], AF.Relu, bias=b21)
                            nc.vector.tensor_scalar(
                                x22[:], z22[:], b22, 0.0, ALU.add, ALU.max)

                        # heads: activations-stationary -> batch-major PSUM
                        for cb in range(4):
                            jl = 16 * h + 4 * cb + jj
                            xs21 = x21[:, 128 * cb:128 * (cb + 1)]
                            xs22 = x22[:, 128 * cb:128 * (cb + 1)]
                            nc.tensor.matmul(
                                ps[:, 4 * jl:4 * jl + 2], xs21, w31,
                                start=True, stop=True)
                            nc.tensor.matmul(
                                ps[:, 4 * jl + 2:4 * jl + 4], xs22, w32,
                                start=True, stop=True)

                    # ---- QP epilogue, batch-major on DVE ----
                    PS = epp.tile([P, 4 * JC], f32, tag="PS")
                    nc.vector.tensor_copy(PS[:], ps[:])
                    ps4 = PS[:].rearrange("p (j k) -> p j k", k=4)
                    Pp = ps4[:, :, 0:2]           # [128, 32, 2]
                    Ss0 = ps4[:, :, 2]            # [128, 32]
                    x01 = xc[:].bitcast(f32).rearrange("p (j f) -> p j f", f=F)[:, :, 0:2]

                    d_t = epp.tile([P, JC, 2], f32, tag="d_t")
                    pp_t = epp.tile([P, JC, 2], f32, tag="pp_t")
                    dsq = epp.tile([P, JC, 2], f32, tag="dsq")
                    dp = epp.tile([P, JC, 2], f32, tag="dp")
                    lam2p = epp.tile([P, JC, 2], f32, tag="lam2p")
                    u_t = epp.tile([P, JC, 2], f32, tag="u_t")
                    sp0 = epp.tile([P, JC], f32, tag="sp0")
                    sig0 = epp.tile([P, JC], f32, tag="sig0")
                    bar0 = epp.tile([P, JC], f32, tag="bar0")
                    bar = epp.tile([P, JC], f32, tag="bar")
                    v1 = epp.tile([P, JC], f32, tag="v1")
                    hb = epp.tile([P, JC], f32, tag="hb")
                    hb2 = epp.tile([P, JC], f32, tag="hb2")
                    viol2 = epp.tile([P, JC], f32, tag="viol2")
                    rv = epp.tile([P, JC], f32, tag="rv")
                    ggq = epp.tile([P, JC], f32, tag="ggq")
                    rec = epp.tile([P, JC], f32, tag="rec")
                    lam2 = epp.tile([P, JC], f32, tag="lam2")
                    ua = epp.tile([P, JC, 2], f32, tag="ua")

                    V = nc.vector
                    V.tensor_tensor(d_t[:], x01, es01, ALU.mult)
                    V.tensor_tensor(d_t[:], d_t[:], em01, ALU.add)
                    V.tensor_tensor(dsq[:], d_t[:], d_t[:], ALU.mult)
                    V.tensor_tensor(pp_t[:], Pp, eb31, ALU.add)
                    V.tensor_tensor(dp[:], d_t[:], pp_t[:], ALU.mult)
                    V.tensor_tensor(sp0[:], Ss0, eb32, ALU.add)
                    nc.scalar.activation(sig0[:], sp0[:], AF.Sigmoid)
                    V.tensor_tensor(bar0[:], dsq[:, :, 0], dsq[:, :, 1], ALU.add)
                    V.tensor_tensor(v1[:], dp[:, :, 0], dp[:, :, 1], ALU.add)
                    V.tensor_scalar(bar[:], bar0[:], -RADIUS * RADIUS, None, ALU.add)
                    V.tensor_tensor(hb[:], sig0[:], bar[:], ALU.mult)
                    V.tensor_scalar(hb2[:], hb[:], 2.0, None, ALU.mult)
                    V.tensor_tensor(viol2[:], v1[:], hb2[:], ALU.subtract)
                    V.tensor_scalar(rv[:], viol2[:], 0.0, None, ALU.max)
                    V.tensor_scalar(ggq[:], bar0[:], EPS / 4.0, None, ALU.add)
                    V.reciprocal(rec[:], ggq[:])
                    V.tensor_tensor(lam2[:], rv[:], rec[:], ALU.mult)
                    V.tensor_copy(lam2p[:, :, 0], lam2[:])
                    V.tensor_copy(lam2p[:, :, 1], lam2[:])
                    V.tensor_tensor(u_t[:], d_t[:], lam2p[:], ALU.mult)
                    V.tensor_tensor(ua[:], u_t[:], pp_t[:], ALU.subtract)

                    nc.sync.dma_start(out_r[:, q * JC:(q + 1) * JC, :], ua[:])

    nc.compile()
    return nc


def make_consts(mean, std, w1, b1, w21, b21, w22, b22, w31, b31, w32, b32):
    cst = np.zeros((P, CW), dtype=np.float32)
    cst[:, C_LT2:C_LT2 + H2] = w21.T
    cst[:, C_LT3:C_LT3 + H2] = w22.T
    cst[:, C_ID:C_ID + 128] = np.eye(128, dtype=np.float32)
    cst[0:F, C_LT1:C_LT1 + H1] = (w1 * std[None, :]).T
    cst[:, C_W31:C_W31 + 2] = w31.T
    cst[:, C_W32:C_W32 + 2] = w32.T
    cst[:, C_B1] = b1 + w1 @ mean
    cst[:, C_B21] = b21
    cst[:, C_B22] = b22
    cst[:, C_EB31:C_EB31 + 2 * JC] = np.tile(b31, JC)[None, :]
    m01 = mean[0:2] - np.array([OBS_X, OBS_Y], dtype=np.float32)
    cst[:, C_EM01:C_EM01 + 2 * JC] = np.tile(m01, JC)[None, :]
    cst[:, C_ES01:C_ES01 + 2 * JC] = np.tile(std[0:2], JC)[None, :]
    cst[:, C_EB32:C_EB32 + JC] = np.float32(b32[0])
    return cst


_PROGRAM_CACHE = {}


def get_program(repeat=1):
    if repeat not in _PROGRAM_CACHE:
        _PROGRAM_CACHE[repeat] = build_program(repeat)
    return _PROGRAM_CACHE[repeat]


def run_on_cores(nc, x_full, cst):
    x_full = np.ascontiguousarray(x_full, dtype=np.float32)
    in_maps = [
        {"x": x_full[c * BS:(c + 1) * BS], "consts": cst}
        for c in range(N_CORES)
    ]
    res = run_bass_kernel_spmd(nc, in_maps, core_ids=list(range(N_CORES)))
    return np.concatenate([res.results[c]["out"] for c in range(N_CORES)], axis=0)


def kernel(x, mean, std, w1, b1, w21, b21, w22, b22, w31, b31, w32, b32, sgn=1,
           **_unused):
    cst = make_consts(
        np.asarray(mean, np.float32), np.asarray(std, np.float32),
        np.asarray(w1, np.float32), np.asarray(b1, np.float32),
        np.asarray(w21, np.float32), np.asarray(b21, np.float32),
        np.asarray(w22, np.float32), np.asarray(b22, np.float32),
        np.asarray(w31, np.float32), np.asarray(b31, np.float32),
        np.asarray(w32, np.float32), np.asarray(b32, np.float32))
    nc = get_program(repeat=1)
    return run_on_cores(nc, np.asarray(x), cst)


# revision 15
# speedup vs baseline: 3.1725x; 3.1725x over previous
"""v2: instruction-count-minimized kernel for the axon-tunneled trn2 backend.

The execution backend charges ~34us per compute instruction regardless of
size; DMA instructions are cheap. So: fewest, widest compute instructions.

Per core (32768 samples), per super-chunk SC of 4096 samples:
  - xt [8, 4096] fm tile loaded by ONE strided gather DMA (col = global row)
  - L1 fp32 8x [128,512]-moving matmuls -> P [128, 4096] (all 8 psum banks)
  - relu+b1 in ONE ACT op [128, 4096] -> h1 bf16
  - L2/L3 bf16 4x [128,1024]-moving matmuls each, wide relu -> x21/x22 bf16
  - L4/L5 heads: bf16 matmuls -> P[0:2, :] and P[32:34, :] (fm)
  - one [34, 4096] PSUM->SBUF copy, two cheap DMAs park heads in DRAM
Epilogue (whole core at once):
  - one DMA gathers heads DRAM -> bm32 [32, (1024, 4)] (sample s at
    partition s%32), one DMA gathers x[:, 0:2] -> [32, (1024, 2)]
  - ~20 wide DVE/ACT ops compute the closed-form QP
  - one DMA scatters u back to out[g, 0:2]
"""

import os
import sys

import numpy as np

for _p in ("/opt/trn_rl_repo", os.path.expanduser("~/.axon_site/_ro/trn_rl_repo")):
    if os.path.isdir(_p) and _p not in sys.path:
        sys.path.append(_p)

import concourse.bacc as bacc
import concourse.mybir as mybir
import concourse.tile as tile
from concourse.bass_utils import run_bass_kernel_spmd

dt = mybir.dt
AF = mybir.ActivationFunctionType
ALU = mybir.AluOpType

N_CORES = 8
B_FULL, F, H1, H2, C = 262144, 8, 128, 128, 2
BS = B_FULL // N_CORES    # 32768 per core
P = 128
SC = 4096                 # samples per super-chunk
NSC = BS // SC            # 8
OBS_X, OBS_Y, RADIUS = 40.0, 15.0, 6.0
EPS = 1e-12

# fp32 const block [128, CW32]
C_LT1 = 0      # (w1*std).T in rows 0:8   [8, 128]
C_B1 = 128     # b1 + w1@mean             [128, 1]
C_B21 = 129    # b21                      [128, 1]
C_B22 = 130    # b22                      [128, 1]
C_S0 = 131     # scalar std[0] broadcast  [128, 1]
C_S1 = 132     # std[1]
C_M0 = 133     # mean[0] - OBS_X
C_M1 = 134     # mean[1] - OBS_Y
C_B31A = 135   # b31[0]
C_B31B = 136   # b31[1]
C_B32A = 137   # b32[0]
CW32 = 398
C_LT2 = 138    # w21.T [128, 128]
C_LT3 = 266    # w22.T [128, 128]
C_W31 = 394    # w31.T [128, 2]
C_W32 = 396    # w32.T [128, 2]



def build_program(repeat=1):
    nc = bacc.Bacc("TRN2", target_bir_lowering=False, debug=False,
                   num_devices=N_CORES)
    f32 = dt.float32

    x_d = nc.dram_tensor("x", [BS, F], f32, kind="ExternalInput").ap()
    c32_d = nc.dram_tensor("c32", [P, CW32], f32, kind="ExternalInput").ap()
    out_d = nc.dram_tensor("out", [BS, C], f32, kind="ExternalOutput").ap()
    hb_d = nc.dram_tensor("hb", [4, BS], f32).ap()  # heads parked fm in DRAM

    with tile.TileContext(nc) as tc:
        with (
            tc.tile_pool(name="cst", bufs=1) as cstp,
            tc.tile_pool(name="xt", bufs=1) as xtp,
            tc.tile_pool(name="act", bufs=1) as actp,
            tc.tile_pool(name="hd", bufs=1) as hdp,
            tc.tile_pool(name="ep", bufs=1) as epp,
            tc.tile_pool(name="ps", bufs=1, space="PSUM") as psp,
        ):
            c32 = cstp.tile([P, CW32], f32)
            nc.sync.dma_start(c32[:], c32_d[:])

            lt1 = c32[0:F, C_LT1:C_LT1 + H1]
            b1 = c32[:, C_B1:C_B1 + 1]
            b21 = c32[:, C_B21:C_B21 + 1]
            b22 = c32[:, C_B22:C_B22 + 1]
            lt2 = c32[:, C_LT2:C_LT2 + H2]
            lt3 = c32[:, C_LT3:C_LT3 + H2]
            w31 = c32[:, C_W31:C_W31 + 2]
            w32 = c32[:, C_W32:C_W32 + 2]

            for _rep in range(repeat):
                for sc in range(NSC):
                    g0 = sc * SC
                    # fm gather: xt[f, c] = x[g0 + c, f]
                    xt = xtp.tile([F, SC], f32, tag="xt")
                    nc.sync.dma_start(
                        xt[:], x_d[g0:g0 + SC, :].rearrange("g f -> f g"))

                    Ppre = psp.tile([P, SC], f32, tag="P")  # all 8 banks
                    for b in range(8):
                        nc.tensor.matmul(Ppre[:, 512 * b:512 * (b + 1)],
                                         lt1, xt[:, 512 * b:512 * (b + 1)],
                                         start=True, stop=True)
                    h1 = actp.tile([P, SC], f32, tag="h1")
                    nc.vector.tensor_scalar(h1[:], Ppre[:], b1, 0.0,
                                            ALU.add, ALU.max)

                    Ppre2 = psp.tile([P, SC], f32, tag="P")
                    for b in range(8):
                        nc.tensor.matmul(Ppre2[:, 512 * b:512 * (b + 1)],
                                         lt2, h1[:, 512 * b:512 * (b + 1)],
                                         start=True, stop=True)
                    x21 = actp.tile([P, SC], f32, tag="x21")
                    nc.vector.tensor_scalar(x21[:], Ppre2[:], b21, 0.0,
                                            ALU.add, ALU.max)

                    Ppre3 = psp.tile([P, SC], f32, tag="P")
                    for b in range(8):
                        nc.tensor.matmul(Ppre3[:, 512 * b:512 * (b + 1)],
                                         lt3, h1[:, 512 * b:512 * (b + 1)],
                                         start=True, stop=True)
                    x22 = actp.tile([P, SC], f32, tag="x22")
                    nc.vector.tensor_scalar(x22[:], Ppre3[:], b22, 0.0,
                                            ALU.add, ALU.max)

                    PH = psp.tile([P, SC], f32, tag="P")
                    for b in range(8):
                        nc.tensor.matmul(PH[0:2, 512 * b:512 * (b + 1)],
                                         w31, x21[:, 512 * b:512 * (b + 1)],
                                         start=True, stop=True)
                    for b in range(8):
                        nc.tensor.matmul(PH[32:34, 512 * b:512 * (b + 1)],
                                         w32, x22[:, 512 * b:512 * (b + 1)],
                                         start=True, stop=True)
                    hs = hdp.tile([34, SC], f32, tag="hs")
                    nc.vector.tensor_copy(hs[:], PH[0:34, :])
                    nc.sync.dma_start(hb_d[0:2, g0:g0 + SC], hs[0:2, :])
                    nc.sync.dma_start(hb_d[2:4, g0:g0 + SC], hs[32:34, :])

                # ---- epilogue: whole core, batch-major-32 ----
                NQ = BS // 32  # 1024 columns per partition-row group
                hbm = epp.tile([32, NQ, 4], f32, tag="hbm")
                # hbm[m, n, k] = hb[k, 32n + m]
                hbv = hb_d.rearrange("k (n m) -> k m n", m=32)
                for k in range(4):
                    nc.sync.dma_start(hbm[:, :, k], hbv[k])
                xb = epp.tile([32, NQ, 2], f32, tag="xb")
                xbv = x_d[:, 0:2].rearrange("(n m) c -> c m n", m=32)
                for k in range(2):
                    nc.sync.dma_start(xb[:, :, k], xbv[k])

                pp = epp.tile([32, NQ, 2], f32, tag="pp")
                d_t = epp.tile([32, NQ, 2], f32, tag="d_t")
                dsq = epp.tile([32, NQ, 2], f32, tag="dsq")
                dp = epp.tile([32, NQ, 2], f32, tag="dp")
                sp0 = epp.tile([32, NQ], f32, tag="sp0")
                sig0 = epp.tile([32, NQ], f32, tag="sig0")
                bar0 = epp.tile([32, NQ], f32, tag="bar0")
                bar = epp.tile([32, NQ], f32, tag="bar")
                v1 = epp.tile([32, NQ], f32, tag="v1")
                hb2 = epp.tile([32, NQ], f32, tag="hb2")
                viol2 = epp.tile([32, NQ], f32, tag="viol2")
                ggq = epp.tile([32, NQ], f32, tag="ggq")
                rec = epp.tile([32, NQ], f32, tag="rec")
                lam2 = epp.tile([32, NQ], f32, tag="lam2")
                u_t = epp.tile([32, NQ, 2], f32, tag="u_t")

                V = nc.vector
                s0c = c32[0:32, C_S0:C_S0 + 1]
                s1c = c32[0:32, C_S1:C_S1 + 1]
                m0c = c32[0:32, C_M0:C_M0 + 1]
                m1c = c32[0:32, C_M1:C_M1 + 1]
                b31a = c32[0:32, C_B31A:C_B31A + 1]
                b31b = c32[0:32, C_B31B:C_B31B + 1]
                b32a = c32[0:32, C_B32A:C_B32A + 1]

                # d = x01 * std01 + (mean01 - obs)
                V.tensor_scalar(d_t[:, :, 0], xb[:, :, 0], s0c, None, ALU.mult)
                V.tensor_scalar(d_t[:, :, 0], d_t[:, :, 0], m0c, None, ALU.add)
                V.tensor_scalar(d_t[:, :, 1], xb[:, :, 1], s1c, None, ALU.mult)
                V.tensor_scalar(d_t[:, :, 1], d_t[:, :, 1], m1c, None, ALU.add)
                V.tensor_tensor(dsq[:], d_t[:], d_t[:], ALU.mult)
                # p' = P + b31
                V.tensor_scalar(pp[:, :, 0], hbm[:, :, 0], b31a, None, ALU.add)
                V.tensor_scalar(pp[:, :, 1], hbm[:, :, 1], b31b, None, ALU.add)
                V.tensor_tensor(dp[:], d_t[:], pp[:], ALU.mult)
                # s'0 = S0 + b32[0]; sig0 = sigmoid(s'0)
                V.tensor_scalar(sp0[:], hbm[:, :, 2], b32a, None, ALU.add)
                nc.scalar.activation(sig0[:], sp0[:], AF.Sigmoid)
                V.tensor_tensor(bar0[:], dsq[:, :, 0], dsq[:, :, 1], ALU.add)
                V.tensor_tensor(v1[:], dp[:, :, 0], dp[:, :, 1], ALU.add)
                V.tensor_scalar(bar[:], bar0[:], -RADIUS * RADIUS, None, ALU.add)
                V.tensor_tensor(hb2[:], sig0[:], bar[:], ALU.mult)
                V.tensor_scalar(hb2[:], hb2[:], 2.0, None, ALU.mult)
                V.tensor_tensor(viol2[:], v1[:], hb2[:], ALU.subtract)
                # lam2 = relu(viol2) / (bar0 + eps/4)
                V.tensor_scalar(viol2[:], viol2[:], 0.0, None, ALU.max)
                V.tensor_scalar(ggq[:], bar0[:], EPS / 4.0, None, ALU.add)
                V.reciprocal(rec[:], ggq[:])
                V.tensor_tensor(lam2[:], viol2[:], rec[:], ALU.mult)
                # u = d * lam2 - p'
                V.tensor_tensor(u_t[:, :, 0], d_t[:, :, 0], lam2[:], ALU.mult)
                V.tensor_tensor(u_t[:, :, 1], d_t[:, :, 1], lam2[:], ALU.mult)
                V.tensor_tensor(u_t[:], u_t[:], pp[:], ALU.subtract)

                outv = out_d.rearrange("(n m) c -> c m n", m=32)
                for k in range(2):
                    nc.sync.dma_start(outv[k], u_t[:, :, k])

    nc.compile()
    return nc


def make_consts(mean, std, w1, b1, w21, b21, w22, b22, w31, b31, w32, b32):
    c32 = np.zeros((P, CW32), dtype=np.float32)
    c32[0:F, C_LT1:C_LT1 + H1] = (w1 * std[None, :]).T
    c32[:, C_B1] = b1 + w1 @ mean
    c32[:, C_B21] = b21
    c32[:, C_B22] = b22
    c32[:, C_S0] = std[0]
    c32[:, C_S1] = std[1]
    c32[:, C_M0] = mean[0] - OBS_X
    c32[:, C_M1] = mean[1] - OBS_Y
    c32[:, C_B31A] = b31[0]
    c32[:, C_B31B] = b31[1]
    c32[:, C_B32A] = b32[0]
    c32[:, C_LT2:C_LT2 + H2] = w21.T
    c32[:, C_LT3:C_LT3 + H2] = w22.T
    c32[:, C_W31:C_W31 + 2] = w31.T
    c32[:, C_W32:C_W32 + 2] = w32.T
    return c32


_PROGRAM_CACHE = {}


def get_program(repeat=1):
    if repeat not in _PROGRAM_CACHE:
        _PROGRAM_CACHE[repeat] = build_program(repeat)
    return _PROGRAM_CACHE[repeat]


def run_on_cores(nc, x_full, c32):
    x_full = np.ascontiguousarray(x_full, dtype=np.float32)
    in_maps = [
        {"x": x_full[c * BS:(c + 1) * BS], "c32": c32}
        for c in range(N_CORES)
    ]
    res = run_bass_kernel_spmd(nc, in_maps, core_ids=list(range(N_CORES)))
    return np.concatenate([res.results[c]["out"] for c in range(N_CORES)], axis=0)


def kernel(x, mean, std, w1, b1, w21, b21, w22, b22, w31, b31, w32, b32, sgn=1,
           **_unused):
    c32 = make_consts(
        np.asarray(mean, np.float32), np.asarray(std, np.float32),
        np.asarray(w1, np.float32), np.asarray(b1, np.float32),
        np.asarray(w21, np.float32), np.asarray(b21, np.float32),
        np.asarray(w22, np.float32), np.asarray(b22, np.float32),
        np.asarray(w31, np.float32), np.asarray(b31, np.float32),
        np.asarray(w32, np.float32), np.asarray(b32, np.float32))
    nc = get_program(repeat=1)
    return run_on_cores(nc, np.asarray(x), c32)
